# revision 37
# baseline (speedup 1.0000x reference)
"""GRU-ODE delay cell on 8 Trainium2 NeuronCores (Bass/Tile), fp8 DoubleRow.

Math (per reference):
    x   = x_coeffs[int(t)]                  # [B, I]
    r   = sigmoid([x, h] @ W_r.T)
    z   = sigmoid([x, h] @ W_z.T)
    h~  = tanh([x, r*h] @ W_h.T)
    dh  = (1 - z) * (h~ - h)

Strategy: data-parallel over batch (B=8192 -> 1024 rows/core), weights
replicated, transposed ([feature, batch]) layout throughout.

Precision plan (validated against the reference in fp64 sim):
  - r gate h-part, h~ gate rh-part, and the first k-pair of the z gate
    run as fp8e4 DoubleRow matmuls (2 contraction rows per PE cell ->
    ~2x matmul throughput at the same 216ns/MM issue rate).
  - the rest of the z gate and all x-parts run in fp16 (same PE speed
    as bf16, 10-bit mantissa): dh = (1-z)(h~-h) amplifies z errors by
    |h~-h| (up to ~6), so z mostly cannot take fp8; fp16 makes its
    error negligible and buys budget for the fp8 gates. Sim sweep:
    nz=2 -> 0.01651, nz=4 -> 0.01868, nz=6 -> 0.02307 (fails 2e-2),
    so NZ_F8=4 is the speed/precision optimum.
  - all weights are pre-scaled by 1024 on host; activations keep natural
    scale; every PSUM readout applies scale=1/1024 inside the ACT op.
    (fp8e4 min normal is 2^-6: scaling weights up moves their mass out
    of the subnormal range.)
  - (h~ - h) subtracts the fp16 h (2^-11 relative, negligible); output
    dh is written in fp16 and upcast on host.
  Simulated max-rel-err 0.01651 (bit-exact match with HW) vs 2e-2.

Orientation per core (hidden tile m of 128 rows, batch free dim 1024):
    psum[m, b] += W.T[k_tile, m_slice].T @ act.T[k_tile, b]
    fp16 stages: one 128-row k-subtile per matmul
    fp8 stages:  DoubleRow pair = 2 k-subtiles per matmul via 3D AP
                 [128, 2, cols]
"""

import numpy as np
import ml_dtypes

B, H, I, TMAX = 8192, 1024, 128, 128
NCORES = 8
BC = B // NCORES          # batch rows per core
NT = H // 128             # 8 hidden output tiles
MM_N = 512                # moving free-dim per matmul (one PSUM bank of fp32)
WS = 1024.0               # host-side weight pre-scale (exact power of 2)

# per-gate count of h-side k-subtiles (of 8) computed in fp8 DoubleRow;
# must be even. Rest (and the x subtile) run fp16.
# NZ_F8=4 validated in fp64 sim (rel err 0.01868 vs 2e-2 gate; sim is
# bit-exact with HW on these deterministic inputs).
NR_F8 = 8
NZ_F8 = 4
NH_F8 = 8
NZ16 = 8 - NZ_F8               # z fp16 h-subtiles (stored after x slot)

_F16 = np.float16
_F8 = ml_dtypes.float8_e4m3   # IEEE-ish variant, max +-240 == TRN FP8_EXP4

_cache = {}


def _build_nc():
    import concourse.bacc as bacc
    import concourse.tile as tile
    import concourse.mybir as mybir

    f32 = mybir.dt.float32
    f16 = mybir.dt.float16
    f8 = mybir.dt.float8e4
    AF = mybir.ActivationFunctionType
    DR = mybir.MatmulPerfMode.DoubleRow
    INV = 1.0 / WS

    nc = bacc.Bacc(
        "TRN2",
        target_bir_lowering=False,
        debug=False,
        enable_asserts=False,
        num_devices=NCORES,
    )

    # DRAM layouts mirror the SBUF tile shapes exactly (host pre-packs).
    xT_d = nc.dram_tensor("xT", [128, BC], f16, kind="ExternalInput").ap()
    h16_d = nc.dram_tensor("hT16", [128, 8, BC], f16, kind="ExternalInput").ap()
    h8_d = nc.dram_tensor("hT8", [128, 8, BC], f8, kind="ExternalInput").ap()
    wrx_d = nc.dram_tensor("wrx", [128, H], f16, kind="ExternalInput").ap()
    wr8_d = nc.dram_tensor("wr8", [128, 8, H], f8, kind="ExternalInput").ap()
    wz_d = nc.dram_tensor("wz", [128, 1 + NZ16, H], f16, kind="ExternalInput").ap()
    wz8_d = nc.dram_tensor("wz8", [128, NZ_F8, H], f8, kind="ExternalInput").ap()
    whx_d = nc.dram_tensor("whx", [128, H], f16, kind="ExternalInput").ap()
    wh8_d = nc.dram_tensor("wh8", [128, 8, H], f8, kind="ExternalInput").ap()
    dh_d = nc.dram_tensor("dhT", [NT, 128, BC], f16, kind="ExternalOutput").ap()
    # sink for the PE warm-up matmuls (keeps them from being DCE'd)
    warm_d = nc.dram_tensor("warm", [128, 4], f32, kind="ExternalOutput").ap()

    bhalves = [(j * MM_N, MM_N) for j in range(BC // MM_N)]

    with tile.TileContext(nc) as tc:
        with (
            tc.tile_pool(name="res", bufs=1) as res,
            tc.tile_pool(name="work", bufs=3) as work,
            tc.tile_pool(name="psum", bufs=4, space="PSUM") as psum,
        ):
            # ---- PE warm-up input (memset must precede the warm matmuls).
            # gpsimd is the first engine through the framework preamble, so
            # the memset lands ~0.7us earlier there than on vector. ----
            warm_in = res.tile([128, 512], f16, name="warm_in", tag="warm_in")
            nc.gpsimd.memset(warm_in[:], 0.0)

            # ---- resident loads. dma_start descriptor generation costs
            # ~0.65us on the ISSUING engine and serializes per engine; the
            # DMA queues are FIFO, so ENQUEUE ORDER (gen completion order)
            # is bandwidth priority — later descriptors wait for earlier
            # ones, no interleave throttling needed. Enqueue order here:
            #   wrx, x | wr8a, h8a | wr8b, h8b | h16a, h16b, wz8, wz16,
            #   wh8, whx
            # matching PE need order (r x-stages -> DR pairs 0-1 -> 2-3 ->
            # rh muls + z fp16 rhs -> z tiles 0-1 -> candidate gate). ----
            x_sb = res.tile([128, BC], f16, name="x_sb", tag="x_sb")
            wrx_sb = res.tile([128, H], f16, name="wrx_sb", tag="wrx_sb")
            wr8_sb = res.tile([128, 8, H], f8, name="wr8_sb", tag="wr8_sb")
            h8_sb = res.tile([128, 8, BC], f8, name="h8_sb", tag="h8_sb")
            h16_sb = res.tile([128, 8, BC], f16, name="h16_sb", tag="h16_sb")
            wz_sb = res.tile([128, 1 + NZ16, H], f16, name="wz_sb", tag="wz_sb")
            wz8_sb = res.tile([128, NZ_F8, H], f8, name="wz8_sb", tag="wz8_sb")
            whx_sb = res.tile([128, H], f16, name="whx_sb", tag="whx_sb")
            wh8_sb = res.tile([128, 8, H], f8, name="wh8_sb", tag="wh8_sb")

            # All resident loads go on the sync chain, in PE-need order.
            # The first loads contend with the framework's sem-init DMA
            # storm (~9-13.6us), so wrx/x are split in halves to get the
            # tiles-0-3 x-stages started as early as possible.
            nc.sync.dma_start(wrx_sb[:, 0:512], wrx_d[:, 0:512])
            nc.sync.dma_start(x_sb[:, 0:512], xT_d[:, 0:512])
            nc.sync.dma_start(wrx_sb[:, 512:H], wrx_d[:, 512:H])
            nc.sync.dma_start(x_sb[:, 512:BC], xT_d[:, 512:BC])
            nc.sync.dma_start(h8_sb[:, 0:4, :], h8_d[:, 0:4, :])
            nc.sync.dma_start(wr8_sb[:, :, 0:512], wr8_d[:, :, 0:512])
            nc.sync.dma_start(h8_sb[:, 4:8, :], h8_d[:, 4:8, :])
            nc.sync.dma_start(wr8_sb[:, :, 512:H], wr8_d[:, :, 512:H])
            nc.sync.dma_start(h16_sb[:, 0:4, :], h16_d[:, 0:4, :])
            nc.sync.dma_start(h16_sb[:, 4:8, :], h16_d[:, 4:8, :])
            nc.sync.dma_start(whx_sb[:], whx_d[:])
            nc.sync.dma_start(wh8_sb[:], wh8_d[:])
            nc.sync.dma_start(wz8_sb[:], wz8_d[:])
            nc.sync.dma_start(wz_sb[:], wz_d[:])

            # ---- PE warm-up: keep the PE busy from t0 so the HAM clock
            # gate reaches 2.4 GHz before the first real matmul. The first
            # ~9 matmuls run at ramp clock (~427-609ns each), so 9 of them
            # bridge until x+wrx land (~10.5us). The warm output DMA sits
            # on gpsimd BEHIND the h8 issues so it can't delay them.
            warm_ps = psum.tile([128, 512], f32, name="warm_ps", tag="ps")
            for _ in range(17):
                nc.tensor.matmul(
                    warm_ps[:], warm_in[:, :128], warm_in[:], start=True, stop=True
                )
            warm_sb = res.tile([128, 4], f32, name="warm_sb", tag="warm_sb")
            nc.vector.tensor_copy(warm_sb[:], warm_ps[:, :4])
            nc.gpsimd.dma_start(warm_d[:], warm_sb[:])

            rh8_sb = res.tile([128, 8, BC], f8, name="rh8_sb", tag="rh8_sb")
            rh16_sb = None
            if NH_F8 < 8:
                rh16_sb = res.tile(
                    [128, 8 - NH_F8, BC], f16, name="rh16_sb", tag="rh16_sb"
                )
            # d = (h~ - h) persists until the z gate (computed last) reads it
            d_sb = [
                res.tile([128, BC], f16, name=f"d{k}", tag=f"d{k}")
                for k in range(NT)
            ]

            def gate_x(ps, n, wx, wz16, halves=None, ps_off=0):
                """x-part stage (fp16, always first -> start=True)."""
                cols = slice(n * 128, (n + 1) * 128)
                lhsT = wx[:, cols] if wx is not None else wz16[:, 0, cols]
                for b0, bw in halves or bhalves:
                    nc.tensor.matmul(
                        ps[:, b0 + ps_off : b0 + ps_off + bw],
                        lhsT,
                        x_sb[:, b0 : b0 + bw],
                        start=True,
                        stop=False,
                    )

            def gate_h(ps, n, w8, wz16, nf8, rhs8, rhs16, rhs16_off=0,
                       halves=None, ps_off=0, pairs=None):
                """h-part stages: nf8 k-subtiles as fp8 DoubleRow pairs,
                the rest fp16. Emitted after gate_x (start=False). `pairs`
                restricts to a subset of DR pairs (stop only fires on the
                overall last stage)."""
                cols = slice(n * 128, (n + 1) * 128)
                nstage = nf8 // 2 + (8 - nf8)
                stage = 0
                for p in (pairs if pairs is not None else range(nf8 // 2)):
                    kk = slice(2 * p, 2 * p + 2)
                    stage = p + 1
                    if isinstance(w8, list):
                        off = (n % 2) * 128
                        lhsT8 = w8[n // 2][:, kk, off : off + 128]
                    else:
                        lhsT8 = w8[:, kk, cols]
                    for b0, bw in halves or bhalves:
                        nc.tensor.matmul(
                            ps[:, b0 + ps_off : b0 + ps_off + bw],
                            lhsT8,
                            rhs8[:, kk, b0 : b0 + bw],
                            start=False,
                            stop=(stage == nstage),
                            perf_mode=DR,
                        )
                for k in range(nf8, 8):
                    stage = nf8 // 2 + (k - nf8) + 1
                    lhsT = wz16[:, k - nf8 + 1, cols]
                    rhs = rhs16[:, k - rhs16_off, :]
                    for b0, bw in halves or bhalves:
                        nc.tensor.matmul(
                            ps[:, b0 + ps_off : b0 + ps_off + bw],
                            lhsT,
                            rhs[:, b0 : b0 + bw],
                            start=False,
                            stop=(stage == nstage),
                        )

            def gate_mms(ps, n, wx, w8, wz16, nf8, rhs8, rhs16, rhs16_off=0,
                         halves=None, ps_off=0):
                gate_x(ps, n, wx, wz16, halves, ps_off)
                gate_h(ps, n, w8, wz16, nf8, rhs8, rhs16, rhs16_off, halves,
                       ps_off)

            # ---- r gate ----
            # The first 4 tiles' stages are ordered by DMA arrival, not by
            # tile: x-stages (x+wrx land first), then DR pairs 0-1 across all
            # four tiles (needs only h8[0:4] + wr8 chunks 0-1), then pairs
            # 2-3 (h8[4:8]). This keeps PE demand matched to the ~330GB/s
            # feed so no single stall exceeds the HAM idle window.
            ps_r = {}
            for n in range(4):
                ps_r[n] = psum.tile([128, BC], f32, name="ps_r", tag="ps")
                gate_x(ps_r[n], n, wrx_sb, None)
            # filler warm-ups: absorb the ~1us wait for wr8a/h8a after the
            # x-stage wave so the HAM clock never drops (a drop costs a
            # ~0.5-1us re-ramp on top of the wait itself). They accumulate
            # zeros (warm_in is memset 0) into the live ps_r[3] with
            # start=False — numerically a no-op, and no PSUM pool slot is
            # consumed (warm_ps's slot was already recycled to ps_r[3]).
            for _ in range(5):
                nc.tensor.matmul(
                    ps_r[3][:, 0:512], warm_in[:, :128], warm_in[:],
                    start=False, stop=False
                )
            for n in range(4):
                gate_h(ps_r[n], n, wr8_sb, None, NR_F8, h8_sb, h16_sb,
                       pairs=[0, 1])
            for n in range(4):
                gate_h(ps_r[n], n, wr8_sb, None, NR_F8, h8_sb, h16_sb,
                       pairs=[2, 3])
                r_t = work.tile([128, BC], f16, name="r_t", tag="r_t")
                nc.scalar.activation(r_t[:], ps_r[n][:], AF.Sigmoid, scale=INV)
                nc.vector.tensor_mul(rh8_sb[:, n, :], r_t[:], h16_sb[:, n, :])
            for n in range(4, NT):
                ps = psum.tile([128, BC], f32, name="ps_r", tag="ps")
                gate_mms(ps, n, wrx_sb, wr8_sb, None, NR_F8, h8_sb, h16_sb)
                r_t = work.tile([128, BC], f16, name="r_t", tag="r_t")
                nc.scalar.activation(r_t[:], ps[:], AF.Sigmoid, scale=INV)
                nc.vector.tensor_mul(rh8_sb[:, n, :], r_t[:], h16_sb[:, n, :])

            # ---- candidate gate (before any z work: its weights arrive
            # right after the r prefix + h16, so the PE never waits on the
            # late wz stream). h~ - h is stashed per tile. ----
            for n in range(NT):
                ps = psum.tile([128, BC], f32, name="ps_h", tag="ps")
                gate_mms(
                    ps, n, whx_sb, wh8_sb, wz_sb, NH_F8, rh8_sb, rh16_sb,
                    rhs16_off=NH_F8,
                )
                for b0, bw in bhalves:
                    sl = slice(b0, b0 + bw)
                    ht = work.tile([128, bw], f16, name="ht", tag="ht")
                    nc.scalar.activation(ht[:], ps[:, sl], AF.Tanh, scale=INV)
                    nc.vector.tensor_sub(
                        d_sb[n][:, sl], ht[:], h16_sb[:, n, sl]
                    )

            # ---- z gate (all tiles) + output ----
            # ends the kernel on the short chain sigmoid -> mul -> DMA;
            # the final tile runs in 256-wide chunks to shorten the tail.
            def z2_out(n, b0, bw, ps, ci, ps_off=0, tag=""):
                sl = slice(b0, b0 + bw)
                psl = slice(b0 + ps_off, b0 + ps_off + bw)
                # tag!="" gives the final chunks dedicated buffers so their
                # muls never wait on an earlier chunk's output DMA (pool
                # ring reuse) — that wait sat on the kernel's critical tail
                zm_t = work.tile([128, bw], f16, name="zm_t", tag=f"zm_t{tag}")
                nc.scalar.activation(zm_t[:], ps[:, psl], AF.Sigmoid, scale=-INV)
                o_t = work.tile([128, bw], f16, name="o_t", tag=f"o_t{tag}")
                nc.vector.tensor_mul(o_t[:], zm_t[:], d_sb[n][:, sl])
                eng = [nc.sync, nc.scalar, nc.gpsimd, nc.sync][ci]
                eng.dma_start(dh_d[n][:, sl], o_t[:])

            for n in range(NT - 1):
                ps = psum.tile([128, BC], f32, name="ps_z2", tag="ps")
                gate_mms(ps, n, None, wz8_sb, wz_sb, NZ_F8, h8_sb, h16_sb)
                for ci, (b0, bw) in enumerate(bhalves):
                    z2_out(n, b0, bw, ps, 0)
            # last tile runs quarter-major (each 256-quarter fully
            # accumulated in turn) so each quarter's sigmoid/mul/DMA
            # overlaps the next quarter's matmuls, and the post-last-matmul
            # chain is only one 256-wide sigmoid+mul+DMA deep. Output
            # descriptor-gens alternate sync/gpsimd (scalar keeps the
            # sigmoid chain).
            n = NT - 1
            for qi in range(4):
                b0 = qi * 256
                psq = psum.tile([128, 256], f32, name=f"ps_z3{qi}", tag="ps")
                gate_mms(psq, n, None, wz8_sb, wz_sb, NZ_F8, h8_sb, h16_sb,
                         halves=[(b0, 256)], ps_off=-b0)
                if qi < 3:
                    z2_out(n, b0, 256, psq, [0, 2, 0][qi], ps_off=-b0,
                           tag=f"_fq{qi}")
                else:
                    # split the very last quarter so the post-last-matmul
                    # chain is only 128 wide (sig+mul+gen+xfer ~1.2us)
                    z2_out(n, b0, 128, psq, 2, ps_off=-b0, tag="_fq3a")
                    z2_out(n, b0 + 128, 128, psq, 0, ps_off=-b0, tag="_fq3b")

    nc.compile()
    return nc


def _pack_weights(W_r, W_z, W_h):
    """Host-side packing: transpose, scale by WS=1024, split x/h parts.

    fp16/fp8 casts are value-exact for the power-of-2 scale; fp8 parts are
    clipped to +-240 (TRN FP8_EXP4 max normal).
    """

    def xpart16(W):            # [128, H] fp16: (p, m) = W[m, p] * WS
        return np.ascontiguousarray(W[:, :I].T * WS).astype(_F16)

    def hpart8(W):             # [128, 8, H] fp8: (p, k, m) = W[m, I+128k+p]*WS
        w = np.ascontiguousarray(W[:, I:].T * WS)       # [1024 kh, 1024 m]
        w = w.reshape(8, 128, H).transpose(1, 0, 2)     # [p, k, m]
        return np.clip(np.ascontiguousarray(w), -240.0, 240.0).astype(_F8)

    wz = np.ascontiguousarray(W_z.T * WS)               # [1152, 1024]
    wz = wz.reshape(9, 128, H).transpose(1, 0, 2)       # [p, k(x first), m]
    # keep only the slots the kernel reads: x-part + fp16 h-subtiles >= NZ_F8
    wz16 = np.ascontiguousarray(
        wz[:, [0] + [1 + k for k in range(NZ_F8, 8)], :]
    ).astype(_F16)

    return {
        "wrx": xpart16(W_r),
        "wr8": hpart8(W_r),                             # [128, 8, 1024]
        "wz": wz16,
        "wz8": np.ascontiguousarray(hpart8(W_z)[:, 0:NZ_F8, :]),
        "whx": xpart16(W_h),
        "wh8": hpart8(W_h),
    }


def _prep_core_inputs(x, h, wpacked):
    """Per-core in_maps. x:[B,I] f32, h:[B,H] f32; weights pre-packed."""
    maps = []
    for c in range(NCORES):
        s = slice(c * BC, (c + 1) * BC)
        xT = np.ascontiguousarray(x[s].T).astype(_F16)           # [128, BC]
        hT = np.ascontiguousarray(h[s].T)                        # [H, BC] f32
        hTk = hT.reshape(8, 128, BC).transpose(1, 0, 2)          # [p, k, b]
        hTk = np.ascontiguousarray(hTk)
        m = {
            "xT": xT,
            "hT16": hTk.astype(_F16),
            "hT8": np.clip(hTk, -240.0, 240.0).astype(_F8),
        }
        m.update(wpacked)
        maps.append(m)
    return maps


def _ensure_axon_hooks_importable():
    """bass_utils imports antenv.axon_hooks when tracing is requested; some
    images ship an antenv stub without it. Provide a no-op fallback so a
    stray BASS_TRACE env var can't crash the run."""
    import sys

    try:
        import antenv.axon_hooks  # noqa: F401
    except ImportError:
        import types

        mod = types.ModuleType("antenv.axon_hooks")
        mod.get_axon_ntff_profile_hook = lambda: None
        mod.set_axon_ntff_profile_hook = lambda h: None
        sys.modules["antenv.axon_hooks"] = mod


def kernel(t, h, x_coeffs, W_r, W_z, W_h):
    _ensure_axon_hooks_importable()
    from concourse.bass_utils import run_bass_kernel_spmd

    t = np.asarray(t)
    h = np.asarray(h, dtype=np.float32)
    x_coeffs = np.asarray(x_coeffs)
    W_r = np.asarray(W_r, dtype=np.float32)
    W_z = np.asarray(W_z, dtype=np.float32)
    W_h = np.asarray(W_h, dtype=np.float32)

    t_int = int(np.clip(np.int32(float(t)), 0, x_coeffs.shape[0] - 1))
    x = np.asarray(x_coeffs[t_int], dtype=np.float32)            # [B, I]

    if "nc" not in _cache:
        _cache["nc"] = _build_nc()
    nc = _cache["nc"]

    wpacked = _pack_weights(W_r, W_z, W_h)
    in_maps = _prep_core_inputs(x, h, wpacked)

    import os

    trace = bool(os.environ.get("BASS_TRACE"))
    res = run_bass_kernel_spmd(nc, in_maps, list(range(NCORES)), trace=trace)
    _cache["last_result"] = res

    outs = []
    for c in range(NCORES):
        dhT = res.results[c]["dhT"]                              # [8,128,BC]
        outs.append(np.asarray(dhT, dtype=np.float32).reshape(H, BC))
    dhT_full = np.concatenate(outs, axis=1)                      # [H, B]
    return np.ascontiguousarray(dhT_full.T).astype(np.float32)   # [B, H]

